# revision 9
# baseline (speedup 1.0000x reference)
"""Trainium2 Bass kernel for nn_AttentionBlock (GroupNorm + single-head spatial
self-attention + residual), data-parallel over batch across 8 NeuronCores.

Reference per sample (C=256, H=W=32, N=H*W=1024 tokens, 32 groups):
    q = GN_q(x) @ Wq + bq ; k = GN_k(x) @ Wk + bk ; v = GN_v(x) @ Wv + bv
    att = softmax((q^T k) / sqrt(C)) over keys;  out = x + (att @ v^T) @ Wo + bo

Math folding (host):
  - GroupNorm affines fold into the projection weights; device only computes
    xh = (x - mu_g) * rsqrt(var_g + eps).
  - Scores: with M^T = Wq_eff @ Wk_eff^T (incl 1/sqrt(C)), U = M @ xh + w1,
    s_T[k, q] = sum_c xh[c, k] * U[c, q]; key-constant terms cancel in
    softmax.
  - Output projection folds into the value weights (Wvo = Wv_eff @ Wo), and
    the output bias bo folds into the value-chain bias (softmax rows sum to
    1, so adding bo*SV to the folded value bias adds exactly bo to out).
  - 1/SV unwind folds into the reciprocal: rbc = exp(-ln(colsum) - ln(SV)).

Device schedule (per core: 4 samples, ~1.5 samples pipelined):
  - All attention matmuls in fp8e4m3 DoubleRow ([128, 2, free] pair APs).
  - PSUM (8 banks) split into three tags: hp (1 buf) for AV output channel
    halves, cp (1 buf) for colsum + the tiny GroupNorm combine/broadcast
    matmuls, ps (2 bufs) ping-ponged by the scores tiles and next-sample
    U / V projection tiles.  The ping-pong lets exp(mt) overlap the
    scores matmul of mt+1, keeping the ScalarE exp chain dense.
  - Next-sample GN/xhat/V/U prep is interleaved between scores groups in
    every engine queue; the AV half-1 matmuls and its evictions are deferred
    into the next iteration so they never block the scores ping-pong.
  - Epilogue: rbc = exp(-ln(colsum)-ln(SV)) on ScalarE; t = hp * rbc on DVE;
    o = t + x on GpSimd (steady state) / DVE (drain tail); stores on the
    sync DMA queue, loads on the gpsimd (SWDGE) queue.
"""

import math

import numpy as np
import ml_dtypes

import concourse.bass as bass
import concourse.tile as tile
from concourse import mybir
from concourse.vector_clock import ScopedClock

F32 = mybir.dt.float32
BF16 = mybir.dt.bfloat16
FP8 = mybir.dt.float8e4
AF = mybir.ActivationFunctionType
ALU = mybir.AluOpType
DR = mybir.MatmulPerfMode.DoubleRow

B, C, H, W = 32, 256, 32, 32
N = H * W            # 1024 spatial tokens
G = 32               # groups
GS = C // G          # 8 channels per group
EPS = 1e-5
NCORES = 8
BS = B // NCORES     # 4 samples per core
CT = C // 128        # 2 channel partition-tiles
MT = N // 128        # 8 token partition-tiles
M2 = MT // 2         # 4 token pair-tiles (fp8 DoubleRow)
SM = 256.0           # fp8 scale on the score chain (M, U)
SV = 32.0            # fp8 scale on the value chain (Wv, V)
LNSV = math.log(SV)


def _patch_tile_drain():
    """walrus in this container allows only ONE sync wait per instruction;
    Tile's final drain carries one wait per live logical processor.  Split
    the waits across SP nops."""
    if getattr(tile.TileContext, "_drain_patched", False):
        return

    def _drain_and_barrier(self, tick_clock, wait_clock):
        nc = self.nc
        drain_inst = nc.sync.drain()
        wait_clock.add_sem_waits(
            drain_inst.ins, ScopedClock({None: tick_clock.global_clock})
        )
        si = drain_inst.ins.sync_info
        waits = list(si.on_wait or [])
        if len(waits) > 1:
            si.on_wait = waits[:1]
            for w in waits[1:]:
                nop_inst = nc.sync.nop()
                nop_inst.ins.sync_info = mybir.SyncInfo(on_wait=[w], on_update=[])

        nc.all_engine_barrier()
        assert self.sems is not None
        popped = nc._tile_sem_poison_stack.pop()
        assert popped is self._sem_poison
        nc.clear_and_free_semaphores(list(self.sems.allocated().values()))
        nc.all_engine_barrier()

    tile.TileContext._drain_and_barrier = _drain_and_barrier
    tile.TileContext._drain_patched = True


def _split_multi_waits(nc):
    """Hoist extra sync waits onto same-engine nops placed just before the
    instruction (engines execute their stream in order, so this is
    equivalent); walrus supports a single wait slot per instruction."""
    k = [0]
    for f in nc.m.functions:
        for b in f.blocks:
            insts = list(b.instructions)
            out = []
            changed = False
            for inst in insts:
                si = inst.sync_info
                if si is not None and si.on_wait and len(si.on_wait) > 1:
                    waits = list(si.on_wait)
                    for w in waits[:-1]:
                        nop = mybir.InstNoOp(
                            name=f"waitsplit-{k[0]}", ins=[], outs=[])
                        k[0] += 1
                        nop.engine = inst.engine
                        nop.sync_info = mybir.SyncInfo(
                            on_wait=[w], on_update=[])
                        out.append(nop)
                        nc.register_instruction(nop, overwrite=True)
                    si.on_wait = waits[-1:]
                    changed = True
                out.append(inst)
            if changed:
                lst = b.instructions
                lst.clear()
                lst.extend(out)
    return nc


def build_nc():
    _patch_tile_drain()
    nc = bass.Bass(trn_type="TRN2")

    x_d = nc.dram_tensor("x", [BS, C, N], F32, kind="ExternalInput")
    y_d = nc.dram_tensor("y", [BS, C, N], F32, kind="ExternalOutput")
    mt_d = nc.dram_tensor("mt", [128, 2, C], FP8, kind="ExternalInput")
    wv_d = nc.dram_tensor("wv", [128, 2, C], FP8, kind="ExternalInput")
    w1_d = nc.dram_tensor("w1", [CT, 128, 1], F32, kind="ExternalInput")
    bv_d = nc.dram_tensor("bv4", [128, 4 * C], F32, kind="ExternalInput")
    ag_d = nc.dram_tensor("ag", [CT, 128, G], F32, kind="ExternalInput")
    bg_d = nc.dram_tensor("bg", [CT, G, 128], F32, kind="ExternalInput")

    with tile.TileContext(nc) as tc:
        _emit(nc, tc, x_d, y_d, mt_d, wv_d, w1_d, bv_d, ag_d, bg_d)
    _split_multi_waits(nc)
    return nc


def _emit(nc, tc, x_d, y_d, mt_d, wv_d, w1_d, bv_d, ag_d, bg_d):
    from contextlib import ExitStack
    ctx = ExitStack()
    with ctx:
        singles = ctx.enter_context(tc.tile_pool(name="singles", bufs=1))
        xpool = ctx.enter_context(tc.tile_pool(name="x", bufs=3))
        xhpool = ctx.enter_context(tc.tile_pool(name="xh", bufs=3))
        stpool = ctx.enter_context(tc.tile_pool(name="st", bufs=2))
        upool = ctx.enter_context(tc.tile_pool(name="u", bufs=2))
        vpool = ctx.enter_context(tc.tile_pool(name="v", bufs=2))
        epool = ctx.enter_context(tc.tile_pool(name="e", bufs=2))
        hpool = ctx.enter_context(tc.tile_pool(name="h", bufs=2))
        opool = ctx.enter_context(tc.tile_pool(name="o", bufs=2))
        pps = ctx.enter_context(tc.tile_pool(name="pps", bufs=1, space="PSUM"))

        def ps_tile(name):
            return pps.tile([128, N], F32, tag="ps", bufs=2, name=name)

        def cp_tile(name, shape=None):
            return pps.tile(shape or [128, N], F32, tag="cp", bufs=1,
                            padded_shape=[128, N], name=name)

        def hp_tile(name):
            return pps.tile([128, N], F32, tag="hp", bufs=1, name=name)

        # ---- constants / weights: tiny, go first on the sync queue ----
        eps_sb = singles.tile([128, 1], F32, tag="eps", name="eps")
        nc.vector.memset(eps_sb[:], EPS)
        nlnsv_sb = singles.tile([128, 1], F32, tag="nlnsv", name="nlnsv")
        nc.vector.memset(nlnsv_sb[:], -LNSV)
        actwarm = singles.tile([128, 1], F32, tag="actwarm", name="actwarm")
        nc.scalar.activation(actwarm[:], eps_sb[:], AF.Exp)
        nc.scalar.activation(actwarm[:], actwarm[:], AF.Ln)

        mt_sb = singles.tile([128, 2, C], FP8, tag="mt", name="mt")
        wv_sb = singles.tile([128, 2, C], FP8, tag="wv", name="wv")
        w1_sb = [singles.tile([128, 1], F32, tag=f"w1{t}", name=f"w1{t}")
                 for t in range(CT)]
        bv_sb = singles.tile([128, 4 * C], F32, tag="bv4", name="bv4")
        ag_sb = [singles.tile([128, G], F32, tag=f"ag{t}", name=f"ag{t}")
                 for t in range(CT)]
        bg_sb = [singles.tile([G, 128], F32, tag=f"bg{t}", name=f"bg{t}")
                 for t in range(CT)]
        for t in range(CT):
            nc.sync.dma_start(ag_sb[t][:], ag_d[t])
            nc.sync.dma_start(bg_sb[t][:], bg_d[t])
            nc.sync.dma_start(w1_sb[t][:], w1_d[t])
        nc.sync.dma_start(mt_sb[:], mt_d[:, :, :])
        nc.sync.dma_start(wv_sb[:], wv_d[:, :, :])
        nc.sync.dma_start(bv_sb[:], bv_d[:, :])
        ones_sb = singles.tile([128, 2, 128], FP8, tag="ones", name="ones")
        nc.vector.memset(ones_sb[:], 1.0)

        x_sb = [None] * BS
        xh8 = [None] * BS    # [128, 2, N] fp8 pair layout: c = 128j + p
        u8 = [None] * BS     # [128, 2, N] fp8 (score chain, scaled by SM)
        v8 = [None] * BS     # 2x [128, 2, 2, C] fp8 (value chain, x SV)
        e8 = [None] * BS     # 4x [128, 2, N] fp8 exp(scores)
        gn_stats = [None] * BS
        hps = [None] * BS    # AV output psum halves
        cps = [None] * BS    # colsum psum
        rbcs = [None] * BS
        t_sb = [None] * BS
        o_sb = [None] * BS

        def emit_load_x(s, spread=False):
            x_sb[s] = [xpool.tile([128, N], F32, tag=f"x{t}", name=f"x{t}")
                       for t in range(CT)]
            for t in range(CT):
                for h in range(2):
                    if spread:
                        eng = nc.sync if h == 0 else nc.gpsimd
                    else:
                        eng = nc.sync if t == 0 else nc.gpsimd
                    eng.dma_start(
                        x_sb[s][t][:, h * 512:(h + 1) * 512],
                        x_d[s, t * 128:(t + 1) * 128,
                            h * 512:(h + 1) * 512])

        def emit_gn_stats(s, halves=True):
            # per-channel mean / mean-square on DVE (bn_stats free max 512)
            stats2 = []
            for t in range(CT):
                nh = 2
                st6 = stpool.tile([128, nh, 6], F32, tag=f"st6_{t}",
                                  name=f"st6_{t}")
                for hh in range(nh):
                    w = N // nh
                    nc.vector.bn_stats(
                        out=st6[:, hh, :],
                        in_=x_sb[s][t][:, hh * w:(hh + 1) * w])
                aggr = stpool.tile([128, 2], F32, tag=f"aggr{t}",
                                   name=f"aggr{t}")
                nc.vector.bn_aggr(out=aggr[:], in_=st6[:])
                st2 = stpool.tile([128, 2], F32, tag=f"st2_{t}",
                                  name=f"st2_{t}")
                nc.vector.tensor_copy(st2[:, 0:1], aggr[:, 0:1])
                nc.vector.tensor_scalar(
                    out=st2[:, 1:2], in0=aggr[:, 0:1],
                    scalar1=aggr[:, 0:1], scalar2=aggr[:, 1:2],
                    op0=ALU.mult, op1=ALU.add)
                stats2.append(st2)
            gn_stats[s] = stats2

        def emit_gn_combine_mm(s):
            gps = cp_tile("gps", [G, 2])
            for t in range(CT):
                nc.tensor.matmul(gps[:], ag_sb[t][:], gn_stats[s][t][:],
                                 start=(t == 0), stop=(t == CT - 1))
            return gps

        def emit_gn_murs(s, gps):
            # group mu / rstd on 32 partitions (DVE + ScalarE)
            g2 = stpool.tile([G, 2], F32, tag="g2", name="g2")
            nc.vector.tensor_copy(g2[:], gps[:])
            murs = stpool.tile([G, 2], F32, tag="murs", name="murs")
            nc.vector.tensor_copy(murs[:, 0:1], g2[:, 0:1])
            nv = stpool.tile([G, 1], F32, tag="nv", name="nv")
            nc.vector.tensor_scalar(
                out=nv[:], in0=g2[:, 0:1],
                scalar1=g2[:, 0:1], scalar2=g2[:, 1:2],
                op0=ALU.mult, op1=ALU.subtract)
            lnv = stpool.tile([G, 1], F32, tag="lnv", name="lnv")
            nc.scalar.activation(lnv[:], nv[:], AF.Ln,
                                 bias=eps_sb[0:G, :], scale=-1.0)
            nc.scalar.activation(murs[:, 1:2], lnv[:], AF.Exp, scale=-0.5)
            return murs

        def emit_gn_bcast_mm(s, murs, t):
            bcps = cp_tile("bcps", [128, 2])
            nc.tensor.matmul(bcps[:], bg_sb[t][:], murs[:],
                             start=True, stop=True)
            return bcps

        def emit_gn_mubc(s, bcps, t):
            mubc = stpool.tile([128, 2], F32, tag=f"mubc{t}",
                               name=f"mubc{t}")
            nc.vector.tensor_copy(mubc[:], bcps[:])
            return mubc

        def alloc_xh8(s):
            xh8[s] = xhpool.tile([128, 2, N], FP8, tag="xh8", name="xh8")

        def emit_xhat(s, mubc, t):
            nc.vector.tensor_scalar(
                out=xh8[s][:, t, :], in0=x_sb[s][t][:],
                scalar1=mubc[:, 0:1], scalar2=mubc[:, 1:2],
                op0=ALU.subtract, op1=ALU.mult)

        def emit_v_mm(s, half):
            # V projection for token blocks 4*half .. 4*half+3
            psv = ps_tile(f"psv{half}")
            for q in range(4):
                tb = 4 * half + q
                nc.tensor.matmul(
                    psv[:, q * C:(q + 1) * C],
                    xh8[s][:, :, tb * 128:(tb + 1) * 128],
                    wv_sb[:],
                    start=True, stop=True, perf_mode=DR)
            return psv

        def emit_v_evict(s, psv, half, eng=None):
            if v8[s] is None:
                v8[s] = [None, None]
            v8[s][half] = vpool.tile([128, 2, 2, C], FP8, tag=f"v8_{half}",
                                     name=f"v8_{half}")
            (eng or nc.vector).tensor_tensor(
                out=v8[s][half][:].rearrange("p a b c -> p (a b c)"),
                in0=psv[:], in1=bv_sb[:], op=ALU.add)

        def emit_u_mm(s, ct):
            psu = ps_tile(f"psu{ct}")
            for nch in range(2):
                nc.tensor.matmul(
                    psu[:, nch * 512:(nch + 1) * 512],
                    mt_sb[:, :, ct * 128:(ct + 1) * 128],
                    xh8[s][:, :, nch * 512:(nch + 1) * 512],
                    start=True, stop=True, perf_mode=DR)
            return psu

        def emit_u_evict(s, psu, ct, on_dve):
            if u8[s] is None:
                u8[s] = upool.tile([128, 2, N], FP8, tag="u8", name="u8")
            if on_dve:
                nc.vector.tensor_scalar(
                    out=u8[s][:, ct, :], in0=psu[:],
                    scalar1=w1_sb[ct][:], scalar2=None, op0=ALU.add)
            else:
                nc.scalar.activation(
                    u8[s][:, ct, :], psu[:], AF.Identity,
                    bias=w1_sb[ct][:])

        def emit_score_mm(s, mt):
            ps = ps_tile(f"pss{mt}")
            for nch in range(2):
                nc.tensor.matmul(
                    ps[:, nch * 512:(nch + 1) * 512],
                    xh8[s][:, :, mt * 128:(mt + 1) * 128],
                    u8[s][:, :, nch * 512:(nch + 1) * 512],
                    start=True, stop=True, perf_mode=DR)
            return ps

        def emit_score_exp(s, ps, mt):
            if e8[s] is None:
                e8[s] = [epool.tile([128, 2, N], FP8, tag=f"e8_{m2}",
                                    name=f"e8_{m2}") for m2 in range(M2)]
            nc.scalar.activation(e8[s][mt // 2][:, mt % 2, :], ps[:],
                                 AF.Exp, scale=1.0 / SM)

        def emit_av_group(s, m2, half):
            # colsum (half 0 only) + AV for output channel block `half`
            if half == 0:
                if m2 == 0:
                    cps[s] = cp_tile("cp")
                    hps[s] = [None, None]
                for nch in range(2):
                    nc.tensor.matmul(
                        cps[s][:, nch * 512:(nch + 1) * 512],
                        ones_sb[:],
                        e8[s][m2][:, :, nch * 512:(nch + 1) * 512],
                        start=(m2 == 0), stop=(m2 == M2 - 1),
                        perf_mode=DR)
            if m2 == 0:
                hps[s][half] = hp_tile(f"hp{half}")
            for nch in range(2):
                nc.tensor.matmul(
                    hps[s][half][:, nch * 512:(nch + 1) * 512],
                    v8[s][m2 // 2][:, m2 % 2, :, half * 128:(half + 1) * 128],
                    e8[s][m2][:, :, nch * 512:(nch + 1) * 512],
                    start=(m2 == 0), stop=(m2 == M2 - 1),
                    perf_mode=DR)

        def emit_recip(s, sl=slice(None)):
            # rbc = 1 / (colsum * SV) via exp(-ln() - ln(SV)) on ScalarE
            if rbcs[s] is None:
                rbcs[s] = hpool.tile([128, N], F32, tag="rbc", name="rbc")
            lncs = hpool.tile([128, N], F32, tag="lncs", name="lncs")
            nc.scalar.activation(lncs[:, sl], cps[s][:, sl], AF.Ln)
            nc.scalar.activation(rbcs[s][:, sl], lncs[:, sl], AF.Exp,
                                 scale=-1.0, bias=nlnsv_sb[:])

        def emit_norm(s, half, sl=slice(None)):
            # t = hp * rbc  (DVE; releases the hp psum slot)
            if t_sb[s] is None:
                t_sb[s] = [hpool.tile([128, N], F32, tag=f"t{dt}",
                                      name=f"t{dt}") for dt in range(CT)]
            nc.vector.tensor_tensor(
                out=t_sb[s][half][:, sl], in0=hps[s][half][:, sl],
                in1=rbcs[s][:, sl], op=ALU.mult)

        def emit_resid(s, half, eng, sl=slice(None)):
            # o = t + x  (GpSimd steady state / DVE tail)
            if o_sb[s] is None:
                o_sb[s] = [opool.tile([128, N], F32, tag=f"o{dt}",
                                      name=f"o{dt}") for dt in range(CT)]
            eng.tensor_tensor(
                out=o_sb[s][half][:, sl], in0=t_sb[s][half][:, sl],
                in1=x_sb[s][half][:, sl], op=ALU.add)

        def emit_store(s, half, sl=slice(None), eng=None):
            (eng or nc.sync).dma_start(
                y_d[s, half * 128:(half + 1) * 128, sl],
                o_sb[s][half][:, sl])

        def emit_gn_chain(s):
            # combine -> murs -> bcast -> mubc -> xhat for sample s
            gps = emit_gn_combine_mm(s)
            murs = emit_gn_murs(s, gps)
            alloc_xh8(s)
            for t in range(CT):
                bcps = emit_gn_bcast_mm(s, murs, t)
                mubc = emit_gn_mubc(s, bcps, t)
                emit_xhat(s, mubc, t)

        # ================= prologue =================
        emit_load_x(0, spread=True)
        emit_load_x(1)
        emit_gn_stats(0, halves=True)
        emit_gn_chain(0)
        # V then U; spread evictions across DVE and ScalarE
        psv0 = emit_v_mm(0, 0)
        emit_v_evict(0, psv0, 0, eng=nc.vector)
        psv1 = emit_v_mm(0, 1)
        psu0 = emit_u_mm(0, 0)
        emit_u_evict(0, psu0, 0, on_dve=False)
        emit_v_evict(0, psv1, 1, eng=nc.vector)
        psu1 = emit_u_mm(0, 1)
        emit_u_evict(0, psu1, 1, on_dve=True)
        emit_gn_stats(1)
        emit_gn_chain(1)

        # ================= main loop =================
        # iteration s: scores(s) with V/U prep(s+1) interleaved; deferred
        # AV half-1 + epilogue of s-1; AV half-0 of s; GN+xhat for s+2
        # (emitted late, so xh8 is ready a full period before its V/U).
        for s in range(BS):
            nx = s + 1 if s + 1 < BS else None
            pv = s - 1 if s >= 1 else None
            n2 = s + 2 if s + 2 < BS else None

            if n2 is not None:
                emit_load_x(n2)           # sync+gpsimd DMA queues, early
            # -- scores mt=0,1 --
            pss0 = emit_score_mm(s, 0)
            emit_score_exp(s, pss0, 0)
            pss1 = emit_score_mm(s, 1)
            emit_score_exp(s, pss1, 1)
            # -- deferred AV half 1 of the previous sample (PE filler) --
            if pv is not None:
                for m2 in range(M2):
                    emit_av_group(pv, m2, 1)
                emit_norm(pv, 1)          # DVE early: frees hp slot
            if nx is not None:
                psv0 = emit_v_mm(nx, 0)   # xh8(nx) ready since last iter
            # -- scores mt=2 + AV(s) m2=0 --
            pss = emit_score_mm(s, 2)
            emit_score_exp(s, pss, 2)
            emit_av_group(s, 0, 0)
            if nx is not None:
                emit_v_evict(nx, psv0, 0)
            # -- scores mt=3 --
            pss = emit_score_mm(s, 3)
            emit_score_exp(s, pss, 3)
            if nx is not None:
                psv1 = emit_v_mm(nx, 1)
            # -- scores mt=4 + AV(s) m2=1 --
            pss = emit_score_mm(s, 4)
            emit_score_exp(s, pss, 4)
            emit_av_group(s, 1, 0)
            if nx is not None:
                emit_v_evict(nx, psv1, 1)
                psu0 = emit_u_mm(nx, 0)
            if n2 is not None:
                emit_gn_stats(n2)         # DVE mid-queue; x(n2) landing
            # -- scores mt=5 + AV(s) m2=2 --
            pss = emit_score_mm(s, 5)
            emit_score_exp(s, pss, 5)
            emit_av_group(s, 2, 0)
            if nx is not None:
                emit_u_evict(nx, psu0, 0, on_dve=False)   # ScalarE
                psu1 = emit_u_mm(nx, 1)
            # -- scores mt=6,7 --
            pss = emit_score_mm(s, 6)
            emit_score_exp(s, pss, 6)
            if nx is not None:
                emit_u_evict(nx, psu1, 1, on_dve=True)    # DVE
            if n2 is not None:
                emit_gn_chain(n2)         # tiny PE/Sca ops + late DVE xhat
            pss = emit_score_mm(s, 7)
            emit_score_exp(s, pss, 7)
            # -- AV(s) m2=3 closes colsum + hp half 0 --
            emit_av_group(s, 3, 0)

            # -- epilogue of the previous sample's half 1 --
            if pv is not None:
                emit_resid(pv, 1, nc.gpsimd if pv + 1 < BS else nc.vector)
                emit_store(pv, 1)

            last = (s == BS - 1)
            if not last:
                # recip + norm half 0 (releases cp and hp-half0 slots)
                emit_recip(s)
                emit_norm(s, 0)
                emit_resid(s, 0, nc.gpsimd)
                emit_store(s, 0)
            else:
                # drain tail: halves through every engine
                for nch in range(2):
                    sl = slice(nch * 512, (nch + 1) * 512)
                    emit_recip(s, sl)
                    emit_norm(s, 0, sl)
                    emit_resid(s, 0, nc.vector, sl)
                    emit_store(s, 0, sl, eng=nc.sync)
                for m2 in range(M2):
                    emit_av_group(s, m2, 1)
                for nch in range(2):
                    sl = slice(nch * 512, (nch + 1) * 512)
                    emit_norm(s, 1, sl)
                    emit_resid(s, 1, nc.vector, sl)
                    emit_store(s, 1, sl, eng=nc.gpsimd)


_NC_CACHE = {}


def _get_nc():
    if "nc" not in _NC_CACHE:
        _NC_CACHE["nc"] = build_nc()
    return _NC_CACHE["nc"]


def _pair(a):
    """[C, X] -> [128, 2, X] fp8 pair layout with c = 128*j + p."""
    a = np.asarray(a, np.float32)
    return np.ascontiguousarray(
        a.reshape(2, 128, a.shape[1]).transpose(1, 0, 2))


def _fp8(a):
    return np.clip(np.asarray(a, np.float32),
                   -240, 240).astype(ml_dtypes.float8_e4m3)


def make_in_maps(**inputs):
    f32 = np.float32
    x = np.asarray(inputs["x"], f32).reshape(B, C, N)
    Wq = np.asarray(inputs["Wq"], f32)
    Wk = np.asarray(inputs["Wk"], f32)
    Wv = np.asarray(inputs["Wv"], f32)
    Wo = np.asarray(inputs["Wo"], f32)
    bq = np.asarray(inputs["bq"], f32)
    bv = np.asarray(inputs["bv"], f32)
    bo = np.asarray(inputs["bo"], f32)
    gq_s = np.asarray(inputs["gq_s"], f32)
    gq_b = np.asarray(inputs["gq_b"], f32)
    gk_s = np.asarray(inputs["gk_s"], f32)
    gv_s = np.asarray(inputs["gv_s"], f32)
    gv_b = np.asarray(inputs["gv_b"], f32)
    # bk and gk_b only shift scores uniformly along the softmax axis -> cancel

    inv_sqrt_c = float(C) ** -0.5
    Wq_eff = (gq_s[:, None] * Wq) * inv_sqrt_c
    bq_eff = (gq_b @ Wq + bq) * inv_sqrt_c
    Wk_eff = gk_s[:, None] * Wk
    m_t = (Wq_eff @ Wk_eff.T) * SM       # lhsT for U: [c', c], fp8-scaled
    w1 = (Wk_eff @ bq_eff) * SM          # [c]
    Wv_eff = gv_s[:, None] * Wv
    bv_eff = gv_b @ Wv + bv
    # fold the output projection into the value chain, and the output bias
    # into the value bias (softmax rows sum to one)
    Wvo = Wv_eff @ Wo
    bvo = bv_eff @ Wo + bo

    ag = np.zeros((C, G), f32)
    bg = np.zeros((G, C), f32)
    for c in range(C):
        ag[c, c // GS] = 1.0 / GS
        bg[c // GS, c] = 1.0

    shared = {
        "mt": _fp8(_pair(m_t)),
        "wv": _fp8(_pair(Wvo * SV)),
        "w1": w1.astype(f32).reshape(CT, 128, 1),
        "bv4": np.tile(bvo[None, :] * SV, (128, 4)).astype(f32),
        "ag": np.ascontiguousarray(ag.reshape(CT, 128, G)),
        "bg": np.ascontiguousarray(bg.reshape(G, CT, 128).transpose(1, 0, 2)),
    }
    in_maps = []
    for i in range(NCORES):
        m = dict(shared)
        m["x"] = np.ascontiguousarray(x[i * BS:(i + 1) * BS])
        in_maps.append(m)
    return in_maps


def run_sharded(inputs, trace=False, **kwargs):
    from concourse.bass_utils import run_bass_kernel_spmd
    nc = _get_nc()
    in_maps = make_in_maps(**inputs)
    res = run_bass_kernel_spmd(nc, in_maps, core_ids=list(range(NCORES)),
                               trace=trace, **kwargs)
    outs = [np.asarray(res.results[i]["y"], np.float32) for i in range(NCORES)]
    full = np.concatenate(outs, axis=0).reshape(B, C, H, W)
    return full, res


def kernel(**inputs):
    out, _ = run_sharded(inputs, trace=False)
    return out


# revision 10
# speedup vs baseline: 1.0365x; 1.0365x over previous
"""Trainium2 Bass kernel for nn_AttentionBlock (GroupNorm + single-head spatial
self-attention + residual), data-parallel over batch across 8 NeuronCores.

Reference per sample (C=256, H=W=32, N=H*W=1024 tokens, 32 groups):
    q = GN_q(x) @ Wq + bq ; k = GN_k(x) @ Wk + bk ; v = GN_v(x) @ Wv + bv
    att = softmax((q^T k) / sqrt(C)) over keys;  out = x + (att @ v^T) @ Wo + bo

Math folding (host):
  - GroupNorm affines fold into the projection weights; device only computes
    xh = (x - mu_g) * rsqrt(var_g + eps).
  - Scores: with M^T = Wq_eff @ Wk_eff^T (incl 1/sqrt(C)), U = M @ xh + w1,
    s_T[k, q] = sum_c xh[c, k] * U[c, q]; key-constant terms cancel in
    softmax.
  - Output projection folds into the value weights (Wvo = Wv_eff @ Wo), and
    the output bias bo folds into the value-chain bias (softmax rows sum to
    1, so adding bo*SV to the folded value bias adds exactly bo to out).
  - 1/SV unwind folds into the reciprocal: rbc = exp(-ln(colsum) - ln(SV)).

Device schedule (per core: 4 samples, ~1.5 samples pipelined):
  - All attention matmuls in fp8e4m3 DoubleRow ([128, 2, free] pair APs).
  - PSUM (8 banks) split into three tags: hp (1 buf) for AV output channel
    halves, cp (1 buf) for colsum + the tiny GroupNorm combine/broadcast
    matmuls, ps (2 bufs) ping-ponged by the scores tiles and next-sample
    U / V projection tiles.  The ping-pong lets exp(mt) overlap the
    scores matmul of mt+1, keeping the ScalarE exp chain dense.
  - Next-sample GN/xhat/V/U prep is interleaved between scores groups in
    every engine queue; the AV half-1 matmuls and its evictions are deferred
    into the next iteration so they never block the scores ping-pong.
  - Epilogue: rbc = exp(-ln(colsum)-ln(SV)) on ScalarE; t = hp * rbc on DVE;
    o = t + x on GpSimd (steady state) / DVE (drain tail); stores on the
    sync DMA queue, loads on the gpsimd (SWDGE) queue.
"""

import math

import numpy as np
import ml_dtypes

import concourse.bass as bass
import concourse.tile as tile
from concourse import mybir
from concourse.vector_clock import ScopedClock

F32 = mybir.dt.float32
BF16 = mybir.dt.bfloat16
FP8 = mybir.dt.float8e4
AF = mybir.ActivationFunctionType
ALU = mybir.AluOpType
DR = mybir.MatmulPerfMode.DoubleRow

B, C, H, W = 32, 256, 32, 32
N = H * W            # 1024 spatial tokens
G = 32               # groups
GS = C // G          # 8 channels per group
EPS = 1e-5
NCORES = 8
BS = B // NCORES     # 4 samples per core
CT = C // 128        # 2 channel partition-tiles
MT = N // 128        # 8 token partition-tiles
M2 = MT // 2         # 4 token pair-tiles (fp8 DoubleRow)
SM = 256.0           # fp8 scale on the score chain (M, U)
SV = 32.0            # fp8 scale on the value chain (Wv, V)
LNSV = math.log(SV)


def _patch_tile_drain():
    """walrus in this container allows only ONE sync wait per instruction;
    Tile's final drain carries one wait per live logical processor.  Split
    the waits across SP nops."""
    if getattr(tile.TileContext, "_drain_patched", False):
        return

    def _drain_and_barrier(self, tick_clock, wait_clock):
        nc = self.nc
        drain_inst = nc.sync.drain()
        wait_clock.add_sem_waits(
            drain_inst.ins, ScopedClock({None: tick_clock.global_clock})
        )
        si = drain_inst.ins.sync_info
        waits = list(si.on_wait or [])
        if len(waits) > 1:
            si.on_wait = waits[:1]
            for w in waits[1:]:
                nop_inst = nc.sync.nop()
                nop_inst.ins.sync_info = mybir.SyncInfo(on_wait=[w], on_update=[])

        nc.all_engine_barrier()
        assert self.sems is not None
        popped = nc._tile_sem_poison_stack.pop()
        assert popped is self._sem_poison
        nc.clear_and_free_semaphores(list(self.sems.allocated().values()))
        nc.all_engine_barrier()

    tile.TileContext._drain_and_barrier = _drain_and_barrier
    tile.TileContext._drain_patched = True


def _split_multi_waits(nc):
    """Hoist extra sync waits onto same-engine nops placed just before the
    instruction (engines execute their stream in order, so this is
    equivalent); walrus supports a single wait slot per instruction."""
    k = [0]
    for f in nc.m.functions:
        for b in f.blocks:
            insts = list(b.instructions)
            out = []
            changed = False
            for inst in insts:
                si = inst.sync_info
                if si is not None and si.on_wait and len(si.on_wait) > 1:
                    waits = list(si.on_wait)
                    for w in waits[:-1]:
                        nop = mybir.InstNoOp(
                            name=f"waitsplit-{k[0]}", ins=[], outs=[])
                        k[0] += 1
                        nop.engine = inst.engine
                        nop.sync_info = mybir.SyncInfo(
                            on_wait=[w], on_update=[])
                        out.append(nop)
                        nc.register_instruction(nop, overwrite=True)
                    si.on_wait = waits[-1:]
                    changed = True
                out.append(inst)
            if changed:
                lst = b.instructions
                lst.clear()
                lst.extend(out)
    return nc


def build_nc():
    _patch_tile_drain()
    nc = bass.Bass(trn_type="TRN2")

    x_d = nc.dram_tensor("x", [BS, C, N], F32, kind="ExternalInput")
    y_d = nc.dram_tensor("y", [BS, C, N], F32, kind="ExternalOutput")
    mt_d = nc.dram_tensor("mt", [128, 2, C], FP8, kind="ExternalInput")
    wv_d = nc.dram_tensor("wv", [128, 2, C], FP8, kind="ExternalInput")
    w1_d = nc.dram_tensor("w1", [CT, 128, 1], F32, kind="ExternalInput")
    bv_d = nc.dram_tensor("bv4", [128, 4 * C], F32, kind="ExternalInput")
    ag_d = nc.dram_tensor("ag", [CT, 128, G], F32, kind="ExternalInput")
    bg_d = nc.dram_tensor("bg", [CT, G, 128], F32, kind="ExternalInput")

    with tile.TileContext(nc) as tc:
        _emit(nc, tc, x_d, y_d, mt_d, wv_d, w1_d, bv_d, ag_d, bg_d)
    _split_multi_waits(nc)
    return nc


def _emit(nc, tc, x_d, y_d, mt_d, wv_d, w1_d, bv_d, ag_d, bg_d):
    from contextlib import ExitStack
    ctx = ExitStack()
    with ctx:
        singles = ctx.enter_context(tc.tile_pool(name="singles", bufs=1))
        xpool = ctx.enter_context(tc.tile_pool(name="x", bufs=4))
        xhpool = ctx.enter_context(tc.tile_pool(name="xh", bufs=3))
        stpool = ctx.enter_context(tc.tile_pool(name="st", bufs=4))
        upool = ctx.enter_context(tc.tile_pool(name="u", bufs=2))
        vpool = ctx.enter_context(tc.tile_pool(name="v", bufs=2))
        epool = ctx.enter_context(tc.tile_pool(name="e", bufs=2))
        hpool = ctx.enter_context(tc.tile_pool(name="h", bufs=2))
        opool = ctx.enter_context(tc.tile_pool(name="o", bufs=2))
        pps = ctx.enter_context(tc.tile_pool(name="pps", bufs=1, space="PSUM"))

        def ps_tile(name):
            return pps.tile([128, N], F32, tag="ps", bufs=2, name=name)

        def cp_tile(name, shape=None):
            return pps.tile(shape or [128, N], F32, tag="cp", bufs=1,
                            padded_shape=[128, N], name=name)

        def hp_tile(name):
            return pps.tile([128, N], F32, tag="hp", bufs=1, name=name)

        # ---- constants / weights: tiny, go first on the sync queue ----
        eps_sb = singles.tile([128, 1], F32, tag="eps", name="eps")
        nc.vector.memset(eps_sb[:], EPS)
        nlnsv_sb = singles.tile([128, 1], F32, tag="nlnsv", name="nlnsv")
        nc.vector.memset(nlnsv_sb[:], -LNSV)
        actwarm = singles.tile([128, 1], F32, tag="actwarm", name="actwarm")
        nc.scalar.activation(actwarm[:], eps_sb[:], AF.Exp)
        nc.scalar.activation(actwarm[:], actwarm[:], AF.Ln)

        mt_sb = singles.tile([128, 2, C], FP8, tag="mt", name="mt")
        wv_sb = singles.tile([128, 2, C], FP8, tag="wv", name="wv")
        w1_sb = [singles.tile([128, 1], F32, tag=f"w1{t}", name=f"w1{t}")
                 for t in range(CT)]
        bv_sb = singles.tile([128, 4 * C], F32, tag="bv4", name="bv4")
        ag_sb = [singles.tile([128, G], F32, tag=f"ag{t}", name=f"ag{t}")
                 for t in range(CT)]
        bg_sb = [singles.tile([G, 128], F32, tag=f"bg{t}", name=f"bg{t}")
                 for t in range(CT)]
        for t in range(CT):
            nc.sync.dma_start(ag_sb[t][:], ag_d[t])
            nc.sync.dma_start(bg_sb[t][:], bg_d[t])
            nc.sync.dma_start(w1_sb[t][:], w1_d[t])
        nc.sync.dma_start(mt_sb[:], mt_d[:, :, :])
        nc.sync.dma_start(wv_sb[:], wv_d[:, :, :])
        nc.sync.dma_start(bv_sb[:], bv_d[:, :])
        ones_sb = singles.tile([128, 2, 128], FP8, tag="ones", name="ones")
        nc.vector.memset(ones_sb[:], 1.0)

        x_sb = [None] * BS
        xh8 = [None] * BS    # [128, 2, N] fp8 pair layout: c = 128j + p
        u8 = [None] * BS     # [128, 2, N] fp8 (score chain, scaled by SM)
        v8 = [None] * BS     # 2x [128, 2, 2, C] fp8 (value chain, x SV)
        e8 = [None] * BS     # 4x [128, 2, N] fp8 exp(scores)
        gn_stats = [None] * BS
        hps = [None] * BS    # AV output psum halves
        cps = [None] * BS    # colsum psum
        rbcs = [None] * BS
        t_sb = [None] * BS
        o_sb = [None] * BS
        mubcs = [None] * BS

        def emit_load_x(s, spread=False):
            x_sb[s] = [xpool.tile([128, N], F32, tag=f"x{t}", name=f"x{t}")
                       for t in range(CT)]
            for t in range(CT):
                for h in range(2):
                    if spread:
                        eng = nc.sync if h == 0 else nc.gpsimd
                    else:
                        eng = nc.sync if t == 0 else nc.gpsimd
                    eng.dma_start(
                        x_sb[s][t][:, h * 512:(h + 1) * 512],
                        x_d[s, t * 128:(t + 1) * 128,
                            h * 512:(h + 1) * 512])

        def emit_gn_stats(s, halves=True):
            # per-channel mean / mean-square on DVE (bn_stats free max 512)
            stats2 = []
            for t in range(CT):
                nh = 2
                st6 = stpool.tile([128, nh, 6], F32, tag=f"st6_{t}",
                                  name=f"st6_{t}")
                for hh in range(nh):
                    w = N // nh
                    nc.vector.bn_stats(
                        out=st6[:, hh, :],
                        in_=x_sb[s][t][:, hh * w:(hh + 1) * w])
                aggr = stpool.tile([128, 2], F32, tag=f"aggr{t}",
                                   name=f"aggr{t}")
                nc.vector.bn_aggr(out=aggr[:], in_=st6[:])
                st2 = stpool.tile([128, 2], F32, tag=f"st2_{t}",
                                  name=f"st2_{t}")
                nc.vector.tensor_copy(st2[:, 0:1], aggr[:, 0:1])
                nc.vector.tensor_scalar(
                    out=st2[:, 1:2], in0=aggr[:, 0:1],
                    scalar1=aggr[:, 0:1], scalar2=aggr[:, 1:2],
                    op0=ALU.mult, op1=ALU.add)
                stats2.append(st2)
            gn_stats[s] = stats2

        def emit_gn_combine_mm(s):
            gps = cp_tile("gps", [G, 2])
            for t in range(CT):
                nc.tensor.matmul(gps[:], ag_sb[t][:], gn_stats[s][t][:],
                                 start=(t == 0), stop=(t == CT - 1))
            return gps

        def emit_gn_murs(s, gps):
            # group mu / rstd on 32 partitions (DVE + ScalarE)
            g2 = stpool.tile([G, 2], F32, tag="g2", name="g2")
            nc.vector.tensor_copy(g2[:], gps[:])
            murs = stpool.tile([G, 2], F32, tag="murs", name="murs")
            nc.vector.tensor_copy(murs[:, 0:1], g2[:, 0:1])
            nv = stpool.tile([G, 1], F32, tag="nv", name="nv")
            nc.vector.tensor_scalar(
                out=nv[:], in0=g2[:, 0:1],
                scalar1=g2[:, 0:1], scalar2=g2[:, 1:2],
                op0=ALU.mult, op1=ALU.subtract)
            lnv = stpool.tile([G, 1], F32, tag="lnv", name="lnv")
            nc.scalar.activation(lnv[:], nv[:], AF.Ln,
                                 bias=eps_sb[0:G, :], scale=-1.0)
            nc.scalar.activation(murs[:, 1:2], lnv[:], AF.Exp, scale=-0.5)
            return murs

        def emit_gn_bcast_mm(s, murs, t):
            bcps = cp_tile("bcps", [128, 2])
            nc.tensor.matmul(bcps[:], bg_sb[t][:], murs[:],
                             start=True, stop=True)
            return bcps

        def emit_gn_mubc(s, bcps, t):
            mubc = stpool.tile([128, 2], F32, tag=f"mubc{t}",
                               name=f"mubc{t}")
            nc.vector.tensor_copy(mubc[:], bcps[:])
            return mubc

        def alloc_xh8(s):
            xh8[s] = xhpool.tile([128, 2, N], FP8, tag="xh8", name="xh8")

        def emit_xhat(s, mubc, t):
            nc.vector.tensor_scalar(
                out=xh8[s][:, t, :], in0=x_sb[s][t][:],
                scalar1=mubc[:, 0:1], scalar2=mubc[:, 1:2],
                op0=ALU.subtract, op1=ALU.mult)

        def emit_v_mm(s, half):
            # V projection for token blocks 4*half .. 4*half+3
            psv = ps_tile(f"psv{half}")
            for q in range(4):
                tb = 4 * half + q
                nc.tensor.matmul(
                    psv[:, q * C:(q + 1) * C],
                    xh8[s][:, :, tb * 128:(tb + 1) * 128],
                    wv_sb[:],
                    start=True, stop=True, perf_mode=DR)
            return psv

        def emit_v_evict(s, psv, half, eng=None):
            if v8[s] is None:
                v8[s] = [None, None]
            v8[s][half] = vpool.tile([128, 2, 2, C], FP8, tag=f"v8_{half}",
                                     name=f"v8_{half}")
            (eng or nc.vector).tensor_tensor(
                out=v8[s][half][:].rearrange("p a b c -> p (a b c)"),
                in0=psv[:], in1=bv_sb[:], op=ALU.add)

        def emit_u_mm(s, ct):
            psu = ps_tile(f"psu{ct}")
            for nch in range(2):
                nc.tensor.matmul(
                    psu[:, nch * 512:(nch + 1) * 512],
                    mt_sb[:, :, ct * 128:(ct + 1) * 128],
                    xh8[s][:, :, nch * 512:(nch + 1) * 512],
                    start=True, stop=True, perf_mode=DR)
            return psu

        def emit_u_evict(s, psu, ct, on_dve):
            if u8[s] is None:
                u8[s] = upool.tile([128, 2, N], FP8, tag="u8", name="u8")
            if on_dve:
                nc.vector.tensor_scalar(
                    out=u8[s][:, ct, :], in0=psu[:],
                    scalar1=w1_sb[ct][:], scalar2=None, op0=ALU.add)
            else:
                nc.scalar.activation(
                    u8[s][:, ct, :], psu[:], AF.Identity,
                    bias=w1_sb[ct][:])

        def emit_score_mm(s, mt):
            ps = ps_tile(f"pss{mt}")
            for nch in range(2):
                nc.tensor.matmul(
                    ps[:, nch * 512:(nch + 1) * 512],
                    xh8[s][:, :, mt * 128:(mt + 1) * 128],
                    u8[s][:, :, nch * 512:(nch + 1) * 512],
                    start=True, stop=True, perf_mode=DR)
            return ps

        def emit_score_exp(s, ps, mt):
            if e8[s] is None:
                e8[s] = [epool.tile([128, 2, N], FP8, tag=f"e8_{m2}",
                                    name=f"e8_{m2}") for m2 in range(M2)]
            nc.scalar.activation(e8[s][mt // 2][:, mt % 2, :], ps[:],
                                 AF.Exp, scale=1.0 / SM)

        def emit_av_group(s, m2, half):
            # colsum (half 0 only) + AV for output channel block `half`
            if half == 0:
                if m2 == 0:
                    cps[s] = cp_tile("cp")
                    hps[s] = [None, None]
                for nch in range(2):
                    nc.tensor.matmul(
                        cps[s][:, nch * 512:(nch + 1) * 512],
                        ones_sb[:],
                        e8[s][m2][:, :, nch * 512:(nch + 1) * 512],
                        start=(m2 == 0), stop=(m2 == M2 - 1),
                        perf_mode=DR)
            if m2 == 0:
                hps[s][half] = hp_tile(f"hp{half}")
            for nch in range(2):
                nc.tensor.matmul(
                    hps[s][half][:, nch * 512:(nch + 1) * 512],
                    v8[s][m2 // 2][:, m2 % 2, :, half * 128:(half + 1) * 128],
                    e8[s][m2][:, :, nch * 512:(nch + 1) * 512],
                    start=(m2 == 0), stop=(m2 == M2 - 1),
                    perf_mode=DR)

        def emit_recip(s, sl=slice(None)):
            # rbc = 1 / (colsum * SV) via exp(-ln() - ln(SV)) on ScalarE
            if rbcs[s] is None:
                rbcs[s] = hpool.tile([128, N], F32, tag="rbc", name="rbc")
            lncs = hpool.tile([128, N], F32, tag="lncs", name="lncs")
            nc.scalar.activation(lncs[:, sl], cps[s][:, sl], AF.Ln)
            nc.scalar.activation(rbcs[s][:, sl], lncs[:, sl], AF.Exp,
                                 scale=-1.0, bias=nlnsv_sb[:])

        def emit_norm(s, half, sl=slice(None)):
            # t = hp * rbc  (DVE; releases the hp psum slot)
            if t_sb[s] is None:
                t_sb[s] = [hpool.tile([128, N], F32, tag=f"t{dt}",
                                      name=f"t{dt}") for dt in range(CT)]
            nc.vector.tensor_tensor(
                out=t_sb[s][half][:, sl], in0=hps[s][half][:, sl],
                in1=rbcs[s][:, sl], op=ALU.mult)

        def emit_resid(s, half, eng, sl=slice(None)):
            # o = t + x  (GpSimd steady state / DVE tail)
            if o_sb[s] is None:
                o_sb[s] = [opool.tile([128, N], F32, tag=f"o{dt}",
                                      name=f"o{dt}") for dt in range(CT)]
            eng.tensor_tensor(
                out=o_sb[s][half][:, sl], in0=t_sb[s][half][:, sl],
                in1=x_sb[s][half][:, sl], op=ALU.add)

        def emit_store(s, half, sl=slice(None), eng=None):
            (eng or nc.sync).dma_start(
                y_d[s, half * 128:(half + 1) * 128, sl],
                o_sb[s][half][:, sl])

        def emit_gn_chain(s, with_xhat=True):
            # combine -> murs -> bcast -> mubc (-> xhat) for sample s
            gps = emit_gn_combine_mm(s)
            murs = emit_gn_murs(s, gps)
            mubcs[s] = [None, None]
            for t in range(CT):
                bcps = emit_gn_bcast_mm(s, murs, t)
                mubcs[s][t] = emit_gn_mubc(s, bcps, t)
            if with_xhat:
                alloc_xh8(s)
                for t in range(CT):
                    emit_xhat(s, mubcs[s][t], t)

        # ================= prologue =================
        # all x DMAs + GroupNorm stats/murs/mubc for every sample run here,
        # hidden under the loads; the main loop then has no GN chain at all.
        emit_load_x(0, spread=True)
        emit_load_x(1)
        emit_gn_stats(0)
        emit_gn_chain(0)
        # V then U; spread evictions across DVE and ScalarE
        psv0 = emit_v_mm(0, 0)
        emit_v_evict(0, psv0, 0, eng=nc.vector)
        psv1 = emit_v_mm(0, 1)
        psu0 = emit_u_mm(0, 0)
        emit_u_evict(0, psu0, 0, on_dve=False)
        emit_v_evict(0, psv1, 1, eng=nc.vector)
        psu1 = emit_u_mm(0, 1)
        emit_u_evict(0, psu1, 1, on_dve=True)
        emit_gn_stats(1)
        emit_gn_chain(1)
        emit_load_x(2)
        emit_gn_stats(2)
        emit_gn_chain(2, with_xhat=False)
        emit_load_x(3)
        emit_gn_stats(3)
        emit_gn_chain(3, with_xhat=False)

        # ================= main loop =================
        # iteration s: scores(s) with V/U prep(s+1) interleaved; deferred
        # AV half-1 groups of s-1 spread between the early scores groups;
        # AV half-0 of s; xhat(s+2) early on DVE (mubc precomputed).
        for s in range(BS):
            nx = s + 1 if s + 1 < BS else None
            pv = s - 1 if s >= 1 else None
            n2 = s + 2 if s + 2 < BS else None

            if n2 is not None:
                alloc_xh8(n2)
                for t in range(CT):
                    emit_xhat(n2, mubcs[n2][t], t)
            # -- scores mt=0,1 + deferred AV half 1 of s-1 --
            pss0 = emit_score_mm(s, 0)
            emit_score_exp(s, pss0, 0)
            if pv is not None:
                emit_av_group(pv, 0, 1)
            pss1 = emit_score_mm(s, 1)
            emit_score_exp(s, pss1, 1)
            if pv is not None:
                emit_av_group(pv, 1, 1)
            if nx is not None:
                psv0 = emit_v_mm(nx, 0)   # xh8(nx) ready since last iter
            # -- scores mt=2 + AV(s) m2=0 --
            pss = emit_score_mm(s, 2)
            emit_score_exp(s, pss, 2)
            emit_av_group(s, 0, 0)
            if pv is not None:
                emit_av_group(pv, 2, 1)
            if nx is not None:
                emit_v_evict(nx, psv0, 0)
            # -- scores mt=3 --
            pss = emit_score_mm(s, 3)
            emit_score_exp(s, pss, 3)
            if pv is not None:
                emit_av_group(pv, 3, 1)
                emit_norm(pv, 1)          # frees the hp slot for AV(s)
            if nx is not None:
                psv1 = emit_v_mm(nx, 1)
            # -- scores mt=4 + AV(s) m2=1 --
            pss = emit_score_mm(s, 4)
            emit_score_exp(s, pss, 4)
            emit_av_group(s, 1, 0)
            if nx is not None:
                emit_v_evict(nx, psv1, 1)
                psu0 = emit_u_mm(nx, 0)
            # -- scores mt=5 + AV(s) m2=2 --
            pss = emit_score_mm(s, 5)
            emit_score_exp(s, pss, 5)
            emit_av_group(s, 2, 0)
            if nx is not None:
                emit_u_evict(nx, psu0, 0, on_dve=False)   # ScalarE
                psu1 = emit_u_mm(nx, 1)
            # -- scores mt=6,7 --
            pss = emit_score_mm(s, 6)
            emit_score_exp(s, pss, 6)
            if nx is not None:
                emit_u_evict(nx, psu1, 1, on_dve=True)    # DVE
            pss = emit_score_mm(s, 7)
            emit_score_exp(s, pss, 7)
            # -- AV(s) m2=3 closes colsum + hp half 0 --
            emit_av_group(s, 3, 0)

            # -- epilogue of the previous sample's half 1 --
            if pv is not None:
                emit_resid(pv, 1, nc.gpsimd if pv + 1 < BS else nc.vector)
                emit_store(pv, 1)

            last = (s == BS - 1)
            if not last:
                # recip + norm half 0 (releases cp and hp-half0 slots)
                emit_recip(s)
                emit_norm(s, 0)
                emit_resid(s, 0, nc.gpsimd)
                emit_store(s, 0)
            else:
                # drain tail: halves through every engine
                for nch in range(2):
                    sl = slice(nch * 512, (nch + 1) * 512)
                    emit_recip(s, sl)
                    emit_norm(s, 0, sl)
                    emit_resid(s, 0, nc.vector, sl)
                    emit_store(s, 0, sl, eng=nc.sync)
                for m2 in range(M2):
                    emit_av_group(s, m2, 1)
                for nch in range(2):
                    sl = slice(nch * 512, (nch + 1) * 512)
                    emit_norm(s, 1, sl)
                    emit_resid(s, 1, nc.vector, sl)
                    emit_store(s, 1, sl, eng=nc.gpsimd)


_NC_CACHE = {}


def _get_nc():
    if "nc" not in _NC_CACHE:
        _NC_CACHE["nc"] = build_nc()
    return _NC_CACHE["nc"]


def _pair(a):
    """[C, X] -> [128, 2, X] fp8 pair layout with c = 128*j + p."""
    a = np.asarray(a, np.float32)
    return np.ascontiguousarray(
        a.reshape(2, 128, a.shape[1]).transpose(1, 0, 2))


def _fp8(a):
    return np.clip(np.asarray(a, np.float32),
                   -240, 240).astype(ml_dtypes.float8_e4m3)


def make_in_maps(**inputs):
    f32 = np.float32
    x = np.asarray(inputs["x"], f32).reshape(B, C, N)
    Wq = np.asarray(inputs["Wq"], f32)
    Wk = np.asarray(inputs["Wk"], f32)
    Wv = np.asarray(inputs["Wv"], f32)
    Wo = np.asarray(inputs["Wo"], f32)
    bq = np.asarray(inputs["bq"], f32)
    bv = np.asarray(inputs["bv"], f32)
    bo = np.asarray(inputs["bo"], f32)
    gq_s = np.asarray(inputs["gq_s"], f32)
    gq_b = np.asarray(inputs["gq_b"], f32)
    gk_s = np.asarray(inputs["gk_s"], f32)
    gv_s = np.asarray(inputs["gv_s"], f32)
    gv_b = np.asarray(inputs["gv_b"], f32)
    # bk and gk_b only shift scores uniformly along the softmax axis -> cancel

    inv_sqrt_c = float(C) ** -0.5
    Wq_eff = (gq_s[:, None] * Wq) * inv_sqrt_c
    bq_eff = (gq_b @ Wq + bq) * inv_sqrt_c
    Wk_eff = gk_s[:, None] * Wk
    m_t = (Wq_eff @ Wk_eff.T) * SM       # lhsT for U: [c', c], fp8-scaled
    w1 = (Wk_eff @ bq_eff) * SM          # [c]
    Wv_eff = gv_s[:, None] * Wv
    bv_eff = gv_b @ Wv + bv
    # fold the output projection into the value chain, and the output bias
    # into the value bias (softmax rows sum to one)
    Wvo = Wv_eff @ Wo
    bvo = bv_eff @ Wo + bo

    ag = np.zeros((C, G), f32)
    bg = np.zeros((G, C), f32)
    for c in range(C):
        ag[c, c // GS] = 1.0 / GS
        bg[c // GS, c] = 1.0

    shared = {
        "mt": _fp8(_pair(m_t)),
        "wv": _fp8(_pair(Wvo * SV)),
        "w1": w1.astype(f32).reshape(CT, 128, 1),
        "bv4": np.tile(bvo[None, :] * SV, (128, 4)).astype(f32),
        "ag": np.ascontiguousarray(ag.reshape(CT, 128, G)),
        "bg": np.ascontiguousarray(bg.reshape(G, CT, 128).transpose(1, 0, 2)),
    }
    in_maps = []
    for i in range(NCORES):
        m = dict(shared)
        m["x"] = np.ascontiguousarray(x[i * BS:(i + 1) * BS])
        in_maps.append(m)
    return in_maps


def run_sharded(inputs, trace=False, **kwargs):
    from concourse.bass_utils import run_bass_kernel_spmd
    nc = _get_nc()
    in_maps = make_in_maps(**inputs)
    res = run_bass_kernel_spmd(nc, in_maps, core_ids=list(range(NCORES)),
                               trace=trace, **kwargs)
    outs = [np.asarray(res.results[i]["y"], np.float32) for i in range(NCORES)]
    full = np.concatenate(outs, axis=0).reshape(B, C, H, W)
    return full, res


def kernel(**inputs):
    out, _ = run_sharded(inputs, trace=False)
    return out


# revision 11
# speedup vs baseline: 1.2547x; 1.2106x over previous
"""Trainium2 Bass kernel for nn_AttentionBlock (GroupNorm + single-head spatial
self-attention + residual), data-parallel over batch across 8 NeuronCores.

Reference per sample (C=256, H=W=32, N=H*W=1024 tokens, 32 groups):
    q = GN_q(x) @ Wq + bq ; k = GN_k(x) @ Wk + bk ; v = GN_v(x) @ Wv + bv
    att = softmax((q^T k) / sqrt(C)) over keys;  out = x + (att @ v^T) @ Wo + bo

Math folding (host):
  - GroupNorm affines fold into the projection weights; device only computes
    xh = (x - mu_g) * rsqrt(var_g + eps).
  - Scores: with M^T = Wq_eff @ Wk_eff^T (incl 1/sqrt(C)), U = M @ xh + w1,
    s_T[k, q] = sum_c xh[c, k] * U[c, q]; key-constant terms cancel in
    softmax.
  - Output projection folds into the value weights (Wvo = Wv_eff @ Wo), and
    the output bias bo folds into the value-chain bias (softmax rows sum to
    1, so adding bo*SV to the folded value bias adds exactly bo to out).
  - 1/SV unwind folds into the reciprocal: rbc = exp(-ln(colsum) - ln(SV)).

Device schedule (per core: 4 samples, ~1.5 samples pipelined):
  - All attention matmuls in fp8e4m3 DoubleRow ([128, 2, free] pair APs).
  - PSUM (8 banks) split into three tags: hp (1 buf) for AV output channel
    halves, cp (1 buf) for colsum + the tiny GroupNorm combine/broadcast
    matmuls, ps (2 bufs) ping-ponged by the scores tiles and next-sample
    U / V projection tiles.  The ping-pong lets exp(mt) overlap the
    scores matmul of mt+1, keeping the ScalarE exp chain dense.
  - Next-sample GN/xhat/V/U prep is interleaved between scores groups in
    every engine queue; the AV half-1 matmuls and its evictions are deferred
    into the next iteration so they never block the scores ping-pong.
  - Epilogue: rbc = exp(-ln(colsum)-ln(SV)) on ScalarE; t = hp * rbc on DVE;
    o = t + x on GpSimd (steady state) / DVE (drain tail); stores on the
    sync DMA queue, loads on the gpsimd (SWDGE) queue.
"""

import math

import numpy as np
import ml_dtypes

import concourse.bass as bass
import concourse.tile as tile
from concourse import mybir
from concourse.vector_clock import ScopedClock

F32 = mybir.dt.float32
BF16 = mybir.dt.bfloat16
FP8 = mybir.dt.float8e4
AF = mybir.ActivationFunctionType
ALU = mybir.AluOpType
DR = mybir.MatmulPerfMode.DoubleRow

B, C, H, W = 32, 256, 32, 32
N = H * W            # 1024 spatial tokens
G = 32               # groups
GS = C // G          # 8 channels per group
EPS = 1e-5
NCORES = 8
BS = B // NCORES     # 4 samples per core
CT = C // 128        # 2 channel partition-tiles
MT = N // 128        # 8 token partition-tiles
M2 = MT // 2         # 4 token pair-tiles (fp8 DoubleRow)
SM = 256.0           # fp8 scale on the score chain (M, U)
SV = 32.0            # fp8 scale on the value chain (Wv, V)
LNSV = math.log(SV)


def _patch_tile_drain():
    """walrus in this container allows only ONE sync wait per instruction;
    Tile's final drain carries one wait per live logical processor.  Split
    the waits across SP nops."""
    if getattr(tile.TileContext, "_drain_patched", False):
        return

    def _drain_and_barrier(self, tick_clock, wait_clock):
        nc = self.nc
        drain_inst = nc.sync.drain()
        wait_clock.add_sem_waits(
            drain_inst.ins, ScopedClock({None: tick_clock.global_clock})
        )
        si = drain_inst.ins.sync_info
        waits = list(si.on_wait or [])
        if len(waits) > 1:
            si.on_wait = waits[:1]
            for w in waits[1:]:
                nop_inst = nc.sync.nop()
                nop_inst.ins.sync_info = mybir.SyncInfo(on_wait=[w], on_update=[])

        nc.all_engine_barrier()
        assert self.sems is not None
        popped = nc._tile_sem_poison_stack.pop()
        assert popped is self._sem_poison
        nc.clear_and_free_semaphores(list(self.sems.allocated().values()))
        nc.all_engine_barrier()

    tile.TileContext._drain_and_barrier = _drain_and_barrier
    tile.TileContext._drain_patched = True


def _split_multi_waits(nc):
    """Hoist extra sync waits onto same-engine nops placed just before the
    instruction (engines execute their stream in order, so this is
    equivalent); walrus supports a single wait slot per instruction."""
    k = [0]
    for f in nc.m.functions:
        for b in f.blocks:
            insts = list(b.instructions)
            out = []
            changed = False
            for inst in insts:
                si = inst.sync_info
                if si is not None and si.on_wait and len(si.on_wait) > 1:
                    waits = list(si.on_wait)
                    for w in waits[:-1]:
                        nop = mybir.InstNoOp(
                            name=f"waitsplit-{k[0]}", ins=[], outs=[])
                        k[0] += 1
                        nop.engine = inst.engine
                        nop.sync_info = mybir.SyncInfo(
                            on_wait=[w], on_update=[])
                        out.append(nop)
                        nc.register_instruction(nop, overwrite=True)
                    si.on_wait = waits[-1:]
                    changed = True
                out.append(inst)
            if changed:
                lst = b.instructions
                lst.clear()
                lst.extend(out)
    return nc


def build_nc():
    _patch_tile_drain()
    nc = bass.Bass(trn_type="TRN2")

    x_d = nc.dram_tensor("x", [BS, C, N], F32, kind="ExternalInput")
    y_d = nc.dram_tensor("y", [BS, C, N], F32, kind="ExternalOutput")
    mt_d = nc.dram_tensor("mt", [128, 2, C], FP8, kind="ExternalInput")
    wv_d = nc.dram_tensor("wv", [128, 2, C], FP8, kind="ExternalInput")
    w1_d = nc.dram_tensor("w1", [CT, 128, 1], F32, kind="ExternalInput")
    bv_d = nc.dram_tensor("bv4", [128, 4 * C], F32, kind="ExternalInput")
    ag_d = nc.dram_tensor("ag", [CT, 128, G], F32, kind="ExternalInput")
    bg_d = nc.dram_tensor("bg", [CT, G, 128], F32, kind="ExternalInput")

    with tile.TileContext(nc) as tc:
        _emit(nc, tc, x_d, y_d, mt_d, wv_d, w1_d, bv_d, ag_d, bg_d)
    _split_multi_waits(nc)
    return nc


def _emit(nc, tc, x_d, y_d, mt_d, wv_d, w1_d, bv_d, ag_d, bg_d):
    from contextlib import ExitStack
    ctx = ExitStack()
    with ctx:
        singles = ctx.enter_context(tc.tile_pool(name="singles", bufs=1))
        xpool = ctx.enter_context(tc.tile_pool(name="x", bufs=4))
        xhpool = ctx.enter_context(tc.tile_pool(name="xh", bufs=3))
        stpool = ctx.enter_context(tc.tile_pool(name="st", bufs=4))
        upool = ctx.enter_context(tc.tile_pool(name="u", bufs=2))
        vpool = ctx.enter_context(tc.tile_pool(name="v", bufs=2))
        epool = ctx.enter_context(tc.tile_pool(name="e", bufs=2))
        hpool = ctx.enter_context(tc.tile_pool(name="h", bufs=2))
        opool = ctx.enter_context(tc.tile_pool(name="o", bufs=2))
        pps = ctx.enter_context(tc.tile_pool(name="pps", bufs=1, space="PSUM"))

        def ps_tile(name):
            return pps.tile([128, N], F32, tag="ps", bufs=2, name=name)

        def cp_tile(name, shape=None):
            return pps.tile(shape or [128, N], F32, tag="cp", bufs=1,
                            padded_shape=[128, N], name=name)

        def hp_tile(name):
            return pps.tile([128, N], F32, tag="hp", bufs=1, name=name)

        # ---- constants / weights: tiny, go first on the sync queue ----
        eps_sb = singles.tile([128, 1], F32, tag="eps", name="eps")
        nc.vector.memset(eps_sb[:], EPS)
        nlnsv_sb = singles.tile([128, 1], F32, tag="nlnsv", name="nlnsv")
        nc.vector.memset(nlnsv_sb[:], -LNSV)
        actwarm = singles.tile([128, 1], F32, tag="actwarm", name="actwarm")
        nc.scalar.activation(actwarm[:], eps_sb[:], AF.Exp)
        nc.scalar.activation(actwarm[:], actwarm[:], AF.Ln)

        mt_sb = singles.tile([128, 2, C], FP8, tag="mt", name="mt")
        wv_sb = singles.tile([128, 2, C], FP8, tag="wv", name="wv")
        w1_sb = [singles.tile([128, 1], F32, tag=f"w1{t}", name=f"w1{t}")
                 for t in range(CT)]
        bv_sb = singles.tile([128, 4 * C], F32, tag="bv4", name="bv4")
        ag_sb = [singles.tile([128, G], F32, tag=f"ag{t}", name=f"ag{t}")
                 for t in range(CT)]
        bg_sb = [singles.tile([G, 128], F32, tag=f"bg{t}", name=f"bg{t}")
                 for t in range(CT)]
        for t in range(CT):
            nc.sync.dma_start(ag_sb[t][:], ag_d[t])
            nc.sync.dma_start(bg_sb[t][:], bg_d[t])
            nc.sync.dma_start(w1_sb[t][:], w1_d[t])
        nc.sync.dma_start(mt_sb[:], mt_d[:, :, :])
        nc.sync.dma_start(wv_sb[:], wv_d[:, :, :])
        nc.sync.dma_start(bv_sb[:], bv_d[:, :])
        ones_sb = singles.tile([128, 2, 128], FP8, tag="ones", name="ones")
        nc.vector.memset(ones_sb[:], 1.0)

        x_sb = [None] * BS
        xh8 = [None] * BS    # [128, 2, N] fp8 pair layout: c = 128j + p
        u8 = [None] * BS     # [128, 2, N] fp8 (score chain, scaled by SM)
        v8 = [None] * BS     # 2x [128, 2, 2, C] fp8 (value chain, x SV)
        e8 = [None] * BS     # 4x [128, 2, N] fp8 exp(scores)
        gn_stats = [None] * BS
        hps = [None] * BS    # AV output psum halves
        cps = [None] * BS    # colsum psum
        rbcs = [None] * BS
        praw = [None] * BS
        t_sb = [None] * BS
        o_sb = [None] * BS
        mubcs = [None] * BS

        def emit_load_x(s, spread=False):
            x_sb[s] = [xpool.tile([128, N], F32, tag=f"x{t}", name=f"x{t}")
                       for t in range(CT)]
            for t in range(CT):
                for h in range(2):
                    if spread:
                        eng = nc.sync if h == 0 else nc.gpsimd
                    else:
                        eng = nc.sync if t == 0 else nc.gpsimd
                    eng.dma_start(
                        x_sb[s][t][:, h * 512:(h + 1) * 512],
                        x_d[s, t * 128:(t + 1) * 128,
                            h * 512:(h + 1) * 512])

        def emit_gn_stats(s, halves=True):
            # per-channel mean / mean-square on DVE (bn_stats free max 512)
            stats2 = []
            for t in range(CT):
                nh = 2
                st6 = stpool.tile([128, nh, 6], F32, tag=f"st6_{t}",
                                  name=f"st6_{t}")
                for hh in range(nh):
                    w = N // nh
                    nc.vector.bn_stats(
                        out=st6[:, hh, :],
                        in_=x_sb[s][t][:, hh * w:(hh + 1) * w])
                aggr = stpool.tile([128, 2], F32, tag=f"aggr{t}",
                                   name=f"aggr{t}")
                nc.vector.bn_aggr(out=aggr[:], in_=st6[:])
                st2 = stpool.tile([128, 2], F32, tag=f"st2_{t}",
                                  name=f"st2_{t}")
                nc.vector.tensor_copy(st2[:, 0:1], aggr[:, 0:1])
                nc.vector.tensor_scalar(
                    out=st2[:, 1:2], in0=aggr[:, 0:1],
                    scalar1=aggr[:, 0:1], scalar2=aggr[:, 1:2],
                    op0=ALU.mult, op1=ALU.add)
                stats2.append(st2)
            gn_stats[s] = stats2

        def emit_gn_combine_mm(s):
            gps = cp_tile("gps", [G, 2])
            for t in range(CT):
                nc.tensor.matmul(gps[:], ag_sb[t][:], gn_stats[s][t][:],
                                 start=(t == 0), stop=(t == CT - 1))
            return gps

        def emit_gn_murs(s, gps):
            # group mu / rstd on 32 partitions (DVE + ScalarE)
            g2 = stpool.tile([G, 2], F32, tag="g2", name="g2")
            nc.vector.tensor_copy(g2[:], gps[:])
            murs = stpool.tile([G, 2], F32, tag="murs", name="murs")
            nc.vector.tensor_copy(murs[:, 0:1], g2[:, 0:1])
            nv = stpool.tile([G, 1], F32, tag="nv", name="nv")
            nc.vector.tensor_scalar(
                out=nv[:], in0=g2[:, 0:1],
                scalar1=g2[:, 0:1], scalar2=g2[:, 1:2],
                op0=ALU.mult, op1=ALU.subtract)
            lnv = stpool.tile([G, 1], F32, tag="lnv", name="lnv")
            nc.scalar.activation(lnv[:], nv[:], AF.Ln,
                                 bias=eps_sb[0:G, :], scale=-1.0)
            nc.scalar.activation(murs[:, 1:2], lnv[:], AF.Exp, scale=-0.5)
            return murs

        def emit_gn_bcast_mm(s, murs, t):
            bcps = cp_tile("bcps", [128, 2])
            nc.tensor.matmul(bcps[:], bg_sb[t][:], murs[:],
                             start=True, stop=True)
            return bcps

        def emit_gn_mubc(s, bcps, t):
            mubc = stpool.tile([128, 2], F32, tag=f"mubc{t}",
                               name=f"mubc{t}")
            nc.vector.tensor_copy(mubc[:], bcps[:])
            return mubc

        def alloc_xh8(s):
            xh8[s] = xhpool.tile([128, 2, N], FP8, tag="xh8", name="xh8")

        def emit_xhat(s, mubc, t):
            nc.vector.tensor_scalar(
                out=xh8[s][:, t, :], in0=x_sb[s][t][:],
                scalar1=mubc[:, 0:1], scalar2=mubc[:, 1:2],
                op0=ALU.subtract, op1=ALU.mult)

        def emit_v_mm(s, half):
            # V projection for token blocks 4*half .. 4*half+3
            psv = ps_tile(f"psv{half}")
            for q in range(4):
                tb = 4 * half + q
                nc.tensor.matmul(
                    psv[:, q * C:(q + 1) * C],
                    xh8[s][:, :, tb * 128:(tb + 1) * 128],
                    wv_sb[:],
                    start=True, stop=True, perf_mode=DR)
            return psv

        def emit_v_evict(s, psv, half, eng=None):
            if v8[s] is None:
                v8[s] = [None, None]
            v8[s][half] = vpool.tile([128, 2, 2, C], FP8, tag=f"v8_{half}",
                                     name=f"v8_{half}")
            (eng or nc.vector).tensor_tensor(
                out=v8[s][half][:].rearrange("p a b c -> p (a b c)"),
                in0=psv[:], in1=bv_sb[:], op=ALU.add)

        def emit_u_mm(s, ct):
            psu = ps_tile(f"psu{ct}")
            for nch in range(2):
                nc.tensor.matmul(
                    psu[:, nch * 512:(nch + 1) * 512],
                    mt_sb[:, :, ct * 128:(ct + 1) * 128],
                    xh8[s][:, :, nch * 512:(nch + 1) * 512],
                    start=True, stop=True, perf_mode=DR)
            return psu

        def emit_u_evict(s, psu, ct, on_dve):
            if u8[s] is None:
                u8[s] = upool.tile([128, 2, N], FP8, tag="u8", name="u8")
            if on_dve:
                nc.vector.tensor_scalar(
                    out=u8[s][:, ct, :], in0=psu[:],
                    scalar1=w1_sb[ct][:], scalar2=None, op0=ALU.add)
            else:
                nc.scalar.activation(
                    u8[s][:, ct, :], psu[:], AF.Identity,
                    bias=w1_sb[ct][:])

        def emit_score_mm(s, mt):
            ps = ps_tile(f"pss{mt}")
            for nch in range(2):
                nc.tensor.matmul(
                    ps[:, nch * 512:(nch + 1) * 512],
                    xh8[s][:, :, mt * 128:(mt + 1) * 128],
                    u8[s][:, :, nch * 512:(nch + 1) * 512],
                    start=True, stop=True, perf_mode=DR)
            return ps

        def emit_score_exp(s, ps, mt):
            if e8[s] is None:
                e8[s] = [epool.tile([128, 2, N], FP8, tag=f"e8_{m2}",
                                    name=f"e8_{m2}") for m2 in range(M2)]
            nc.scalar.activation(e8[s][mt // 2][:, mt % 2, :], ps[:],
                                 AF.Exp, scale=1.0 / SM)

        def emit_av_group(s, m2, half):
            # colsum (half 0 only) + AV for output channel block `half`
            if half == 0:
                if m2 == 0:
                    cps[s] = cp_tile("cp")
                    hps[s] = [None, None]
                for nch in range(2):
                    nc.tensor.matmul(
                        cps[s][:, nch * 512:(nch + 1) * 512],
                        ones_sb[:],
                        e8[s][m2][:, :, nch * 512:(nch + 1) * 512],
                        start=(m2 == 0), stop=(m2 == M2 - 1),
                        perf_mode=DR)
            if m2 == 0:
                hps[s][half] = hp_tile(f"hp{half}")
            for nch in range(2):
                nc.tensor.matmul(
                    hps[s][half][:, nch * 512:(nch + 1) * 512],
                    v8[s][m2 // 2][:, m2 % 2, :, half * 128:(half + 1) * 128],
                    e8[s][m2][:, :, nch * 512:(nch + 1) * 512],
                    start=(m2 == 0), stop=(m2 == M2 - 1),
                    perf_mode=DR)

        def emit_recip(s, sl=slice(None)):
            # rbc = 1 / (colsum * SV) via exp(-ln() - ln(SV)) on ScalarE
            if rbcs[s] is None:
                rbcs[s] = hpool.tile([128, N], F32, tag="rbc", name="rbc")
            lncs = hpool.tile([128, N], F32, tag="lncs", name="lncs")
            nc.scalar.activation(lncs[:, sl], cps[s][:, sl], AF.Ln)
            nc.scalar.activation(rbcs[s][:, sl], lncs[:, sl], AF.Exp,
                                 scale=-1.0, bias=nlnsv_sb[:])

        def emit_praw(s, half, on_dve=True):
            # raw PSUM -> SBUF eviction of the AV output: releases the hp
            # slot immediately (no rbc dependency), keeping the PE stream
            # dense across the sample boundary
            if praw[s] is None:
                praw[s] = [hpool.tile([128, N], F32, tag=f"pr{dt}",
                                      name=f"pr{dt}") for dt in range(CT)]
            if on_dve:
                nc.vector.tensor_copy(praw[s][half][:], hps[s][half][:])
            else:
                nc.scalar.activation(praw[s][half][:], hps[s][half][:],
                                     AF.Identity)

        def emit_norm(s, half, sl=slice(None), src=None):
            # t = praw * rbc on SBUF data (or straight from PSUM in the tail)
            if t_sb[s] is None:
                t_sb[s] = [hpool.tile([128, N], F32, tag=f"t{dt}",
                                      name=f"t{dt}") for dt in range(CT)]
            nc.vector.tensor_tensor(
                out=t_sb[s][half][:, sl],
                in0=(praw[s][half] if src is None else src)[:, sl],
                in1=rbcs[s][:, sl], op=ALU.mult)

        def emit_resid(s, half, eng, sl=slice(None)):
            # o = t + x  (GpSimd steady state / DVE tail)
            if o_sb[s] is None:
                o_sb[s] = [opool.tile([128, N], F32, tag=f"o{dt}",
                                      name=f"o{dt}") for dt in range(CT)]
            eng.tensor_tensor(
                out=o_sb[s][half][:, sl], in0=t_sb[s][half][:, sl],
                in1=x_sb[s][half][:, sl], op=ALU.add)

        def emit_store(s, half, sl=slice(None), eng=None):
            (eng or nc.sync).dma_start(
                y_d[s, half * 128:(half + 1) * 128, sl],
                o_sb[s][half][:, sl])

        def emit_gn_chain(s, with_xhat=True):
            # combine -> murs -> bcast -> mubc (-> xhat) for sample s
            gps = emit_gn_combine_mm(s)
            murs = emit_gn_murs(s, gps)
            mubcs[s] = [None, None]
            for t in range(CT):
                bcps = emit_gn_bcast_mm(s, murs, t)
                mubcs[s][t] = emit_gn_mubc(s, bcps, t)
            if with_xhat:
                alloc_xh8(s)
                for t in range(CT):
                    emit_xhat(s, mubcs[s][t], t)

        # ================= prologue =================
        # all x DMAs + GroupNorm stats/murs/mubc for every sample run here,
        # hidden under the loads; the main loop then has no GN chain at all.
        emit_load_x(0, spread=True)
        emit_load_x(1)
        emit_gn_stats(0)
        emit_gn_chain(0)
        # U first (scores gate on it), V overlaps the first scores groups
        psu0 = emit_u_mm(0, 0)
        emit_u_evict(0, psu0, 0, on_dve=False)
        psu1 = emit_u_mm(0, 1)
        emit_u_evict(0, psu1, 1, on_dve=True)
        psv0 = emit_v_mm(0, 0)
        emit_v_evict(0, psv0, 0, eng=nc.vector)
        psv1 = emit_v_mm(0, 1)
        emit_v_evict(0, psv1, 1, eng=nc.vector)
        emit_load_x(2)
        emit_load_x(3)
        emit_gn_stats(1)
        emit_gn_chain(1)
        emit_gn_stats(2)
        emit_gn_chain(2, with_xhat=False)
        emit_gn_stats(3)
        emit_gn_chain(3, with_xhat=False)

        # ================= main loop =================
        # iteration s: scores(s) with V/U prep(s+1) interleaved and AV(s)
        # half-0 groups paced by the exps; after exp7 the PE runs AV m2=3,
        # then AV half-1 densely; raw evictions free the PSUM slots without
        # waiting for the reciprocal, which normalizes SBUF data later.
        for s in range(BS):
            nx = s + 1 if s + 1 < BS else None
            n2 = s + 2 if s + 2 < BS else None
            last = (s == BS - 1)

            if n2 is not None:
                alloc_xh8(n2)
                for t in range(CT):
                    emit_xhat(n2, mubcs[n2][t], t)
            # -- scores mt=0,1 --
            pss0 = emit_score_mm(s, 0)
            emit_score_exp(s, pss0, 0)
            pss1 = emit_score_mm(s, 1)
            emit_score_exp(s, pss1, 1)
            if nx is not None:
                psv0 = emit_v_mm(nx, 0)   # xh8(nx) ready since last iter
            # -- scores mt=2 + AV(s) m2=0 --
            pss = emit_score_mm(s, 2)
            emit_score_exp(s, pss, 2)
            emit_av_group(s, 0, 0)
            if nx is not None:
                emit_v_evict(nx, psv0, 0)
            # -- scores mt=3 --
            pss = emit_score_mm(s, 3)
            emit_score_exp(s, pss, 3)
            if nx is not None:
                psv1 = emit_v_mm(nx, 1)
            # -- scores mt=4 + AV(s) m2=1 --
            pss = emit_score_mm(s, 4)
            emit_score_exp(s, pss, 4)
            emit_av_group(s, 1, 0)
            if nx is not None:
                emit_v_evict(nx, psv1, 1)
                psu0 = emit_u_mm(nx, 0)
            # -- scores mt=5 + AV(s) m2=2 --
            pss = emit_score_mm(s, 5)
            emit_score_exp(s, pss, 5)
            emit_av_group(s, 2, 0)
            if nx is not None:
                emit_u_evict(nx, psu0, 0, on_dve=False)   # ScalarE
                psu1 = emit_u_mm(nx, 1)
            # -- scores mt=6,7 --
            pss = emit_score_mm(s, 6)
            emit_score_exp(s, pss, 6)
            if nx is not None:
                emit_u_evict(nx, psu1, 1, on_dve=True)    # DVE
            pss = emit_score_mm(s, 7)
            emit_score_exp(s, pss, 7)
            # -- AV(s) m2=3 closes colsum + hp half 0; half 1 runs densely --
            emit_av_group(s, 3, 0)
            if not last:
                emit_praw(s, 0, on_dve=True)
                for m2 in range(M2):
                    emit_av_group(s, m2, 1)
                emit_praw(s, 1, on_dve=True)
                emit_recip(s)
                emit_norm(s, 0)
                emit_resid(s, 0, nc.gpsimd)
                emit_store(s, 0)
                emit_norm(s, 1)
                emit_resid(s, 1, nc.gpsimd)
                emit_store(s, 1)
            else:
                # drain tail: normalize straight from PSUM in halves
                for nch in range(2):
                    sl = slice(nch * 512, (nch + 1) * 512)
                    emit_recip(s, sl)
                    emit_norm(s, 0, sl, src=hps[s][0])
                    emit_resid(s, 0, nc.vector, sl)
                    emit_store(s, 0, sl, eng=nc.sync)
                for m2 in range(M2):
                    emit_av_group(s, m2, 1)
                for nch in range(2):
                    sl = slice(nch * 512, (nch + 1) * 512)
                    emit_norm(s, 1, sl, src=hps[s][1])
                    emit_resid(s, 1, nc.vector, sl)
                    emit_store(s, 1, sl, eng=nc.gpsimd)


_NC_CACHE = {}


def _get_nc():
    if "nc" not in _NC_CACHE:
        _NC_CACHE["nc"] = build_nc()
    return _NC_CACHE["nc"]


def _pair(a):
    """[C, X] -> [128, 2, X] fp8 pair layout with c = 128*j + p."""
    a = np.asarray(a, np.float32)
    return np.ascontiguousarray(
        a.reshape(2, 128, a.shape[1]).transpose(1, 0, 2))


def _fp8(a):
    return np.clip(np.asarray(a, np.float32),
                   -240, 240).astype(ml_dtypes.float8_e4m3)


def make_in_maps(**inputs):
    f32 = np.float32
    x = np.asarray(inputs["x"], f32).reshape(B, C, N)
    Wq = np.asarray(inputs["Wq"], f32)
    Wk = np.asarray(inputs["Wk"], f32)
    Wv = np.asarray(inputs["Wv"], f32)
    Wo = np.asarray(inputs["Wo"], f32)
    bq = np.asarray(inputs["bq"], f32)
    bv = np.asarray(inputs["bv"], f32)
    bo = np.asarray(inputs["bo"], f32)
    gq_s = np.asarray(inputs["gq_s"], f32)
    gq_b = np.asarray(inputs["gq_b"], f32)
    gk_s = np.asarray(inputs["gk_s"], f32)
    gv_s = np.asarray(inputs["gv_s"], f32)
    gv_b = np.asarray(inputs["gv_b"], f32)
    # bk and gk_b only shift scores uniformly along the softmax axis -> cancel

    inv_sqrt_c = float(C) ** -0.5
    Wq_eff = (gq_s[:, None] * Wq) * inv_sqrt_c
    bq_eff = (gq_b @ Wq + bq) * inv_sqrt_c
    Wk_eff = gk_s[:, None] * Wk
    m_t = (Wq_eff @ Wk_eff.T) * SM       # lhsT for U: [c', c], fp8-scaled
    w1 = (Wk_eff @ bq_eff) * SM          # [c]
    Wv_eff = gv_s[:, None] * Wv
    bv_eff = gv_b @ Wv + bv
    # fold the output projection into the value chain, and the output bias
    # into the value bias (softmax rows sum to one)
    Wvo = Wv_eff @ Wo
    bvo = bv_eff @ Wo + bo

    ag = np.zeros((C, G), f32)
    bg = np.zeros((G, C), f32)
    for c in range(C):
        ag[c, c // GS] = 1.0 / GS
        bg[c // GS, c] = 1.0

    shared = {
        "mt": _fp8(_pair(m_t)),
        "wv": _fp8(_pair(Wvo * SV)),
        "w1": w1.astype(f32).reshape(CT, 128, 1),
        "bv4": np.tile(bvo[None, :] * SV, (128, 4)).astype(f32),
        "ag": np.ascontiguousarray(ag.reshape(CT, 128, G)),
        "bg": np.ascontiguousarray(bg.reshape(G, CT, 128).transpose(1, 0, 2)),
    }
    in_maps = []
    for i in range(NCORES):
        m = dict(shared)
        m["x"] = np.ascontiguousarray(x[i * BS:(i + 1) * BS])
        in_maps.append(m)
    return in_maps


def run_sharded(inputs, trace=False, **kwargs):
    from concourse.bass_utils import run_bass_kernel_spmd
    nc = _get_nc()
    in_maps = make_in_maps(**inputs)
    res = run_bass_kernel_spmd(nc, in_maps, core_ids=list(range(NCORES)),
                               trace=trace, **kwargs)
    outs = [np.asarray(res.results[i]["y"], np.float32) for i in range(NCORES)]
    full = np.concatenate(outs, axis=0).reshape(B, C, H, W)
    return full, res


def kernel(**inputs):
    out, _ = run_sharded(inputs, trace=False)
    return out
